# revision 13
# baseline (speedup 1.0000x reference)
"""Trainium2 Bass kernel for CARE position encoding (rotor sandwich).

out = R x R~ factorizes into 4 sequential Givens stages (blades 6,9,5,3).
This implementation:
  - computes all cos/sin tables on the HOST (from pos/theta/coefs) and
    ships them as fp16 -- the device does zero transcendental work;
  - stores x per-core in a position-innermost "slot" layout
    X[partition, slot*J + j] (J=256 positions per partition, 14 slots;
    multivector components 0 and 15 are invariant and bypass the device);
  - each Givens stage is 3 (or 6) DVE tensor_tensor ops in fp16, whose
    access patterns have unit-stride 256-long innermost runs -> the DVE
    runs them in 2x_1P packed mode (verified on HW);
  - the slot permutation was chosen so every stage's pair structure is an
    affine "grid" slot(q,e) = s0 + dq*q + de*e expressible in <=3 free
    AP dims (planes 6,3 as one op-triple; planes 9,5 as two halves).

Sign conventions (tau = Cayley sign of the rotated pair) are baked into
per-sub sign tables SS[r*J + j], r = q + nq*e, so arbitrary per-pair
orientations are free.
"""
import numpy as np

import concourse.bass as bass
import concourse.tile as tile
from concourse import bacc, mybir
from concourse.bass_utils import run_bass_kernel_spmd

F16 = mybir.dt.float16
F32 = mybir.dt.float32

P = 128
NCORES = 8
B, L, MV = 16, 16384, 16
MAX_LEN = 16384
ROWS_PER_CORE = B // NCORES          # 2
N = ROWS_PER_CORE * L                # 32768 positions per core
J = N // P                           # 256 positions per partition
NSLOT = 14

PLANE_BLADES = (3, 5, 9, 6)          # reference order (stage order reversed)
STAGE_ORDER = (6, 9, 5, 3)           # innermost rotor applied first

# slot[comp] for comps 1..14 (0 and 15 bypass the device entirely)
SLOT = {1: 3, 2: 13, 3: 9, 4: 6, 5: 2, 6: 12, 7: 5, 8: 8,
        9: 1, 10: 11, 11: 7, 12: 4, 13: 0, 14: 10}
COMPS = [c for c in range(MV) if c not in (0, 15)]
SLOT_TO_COMP = {s: c for c, s in SLOT.items()}

# Per-stage sub-ops: (nq, dq, de, s0, placement) with
# slot(comp placement[q][e]) = s0 + dq*q + de*e ; validated vs Cayley below.
STAGE_SUBS = {
    6: [(4, -2, 7, 6, ((4, 2), (12, 10), (5, 3), (13, 11)))],
    9: [(2, 2, 5, 3, ((1, 8), (7, 14))),
        (2, -7, -2, 11, ((10, 3), (12, 5)))],
    5: [(2, 6, 3, 3, ((1, 4), (3, 6))),
        (2, 6, 3, 1, ((9, 12), (11, 14)))],
    3: [(4, -1, 10, 3, ((1, 2), (5, 6), (9, 10), (13, 14)))],
}

# Table layout (units of J elements per partition), one CC + one shared SS
# per PLANE.  SS rows: m6/m9/m3 uniform-tau -> 2 rows [+s2, -s2]; m5 mixed
# pattern (+,-) per half -> 4 rows [s2, -s2, -s2, s2] (r = 2q + e).
# Stage order: m6 (3J) | m9 (3J) | m5 (5J) | m3 (3J) = 14J total.
_TBL_PLANE = {6: (0, 1, 2), 9: (3, 4, 2), 5: (6, 7, 4), 3: (11, 12, 2)}
TBL_J = 14

# slots 4..9 are final after stage 5 (not touched by stage 3)
EARLY_OUT = (4, 10)                   # slot range [4, 10)
LATE_OUT = ((0, 4), (10, 14))


def _build_cayley(k=4):
    n = 1 << k
    C = np.zeros((n, n, n), dtype=np.float32)
    for a in range(n):
        for b in range(n):
            s, t = 0, a >> 1
            while t:
                s += bin(t & b).count("1")
                t >>= 1
            C[a, b, a ^ b] = -1.0 if (s & 1) else 1.0
    return C


def _verify_layout(cayley):
    """Check SLOT/STAGE_SUBS against the runtime Cayley tensor."""
    for m in STAGE_ORDER:
        rotated = set()
        for (nq, dq, de, s0, placement) in STAGE_SUBS[m]:
            for q, (a, b) in enumerate(placement):
                assert b == (a ^ m), (m, a, b)
                assert SLOT[a] == s0 + dq * q, (m, q, a)
                assert SLOT[b] == s0 + dq * q + de, (m, q, b)
                assert abs(cayley[a, m, b]) == 1.0
                rotated |= {a, b}
        expect = {c for c in range(MV) if bin(c & m).count("1") % 2 == 1}
        assert rotated == expect, (m, rotated, expect)


def _ap(base_ap, extra_off, dims):
    ap = [list(base_ap.ap[0])] + [list(d) for d in dims]
    return bass.AP(base_ap.tensor, base_ap.offset + extra_off, ap)


def _build_program():
    nc = bacc.Bacc("TRN2", target_bir_lowering=False, debug=False,
                   enable_asserts=False, num_devices=NCORES)
    x_d = nc.dram_tensor("x", [P, NSLOT * J], F16, kind="ExternalInput")
    t_d = nc.dram_tensor("tbl", [P, TBL_J * J], F16, kind="ExternalInput")
    out_d = nc.dram_tensor("out", [P, NSLOT * J], F16, kind="ExternalOutput")

    cayley = _build_cayley()

    def ss_ap(TBL, m, sub):
        nq, dq, de, s0, placement = sub
        ss_j = _TBL_PLANE[m][1]
        tau0 = [float(cayley[a, m, b]) for (a, b) in placement]
        if all(t == tau0[0] for t in tau0):
            t = tau0[0]
            off = ss_j * J + (0 if t > 0 else J)
            estep = J if t > 0 else -J
            return _ap(TBL[:], off, [[0, nq], [estep, 2], [1, J]])
        assert nq == 2 and tau0 == [1.0, -1.0], (m, tau0)
        return _ap(TBL[:], ss_j * J, [[2 * J, nq], [J, 2], [1, J]])

    with tile.TileContext(nc) as tc:
        with tc.tile_pool(name="data", bufs=1) as dpool, \
             tc.tile_pool(name="tu", bufs=1) as tupool:
            TBL = dpool.tile([P, TBL_J * J], F16)
            X = dpool.tile([P, NSLOT * J], F16)

            # spread input DMAs across idle engine queues so issues (and,
            # if the rings allow, transfers) run in parallel after the
            # framework preamble barrier
            nc.sync.dma_start(TBL[:, :3 * J], t_d[:, :3 * J])
            nc.sync.dma_start(X[:, :7 * J], x_d[:, :7 * J])
            nc.sync.dma_start(X[:, 7 * J:], x_d[:, 7 * J:])
            nc.sync.dma_start(TBL[:, 3 * J:], t_d[:, 3 * J:])

            for m in STAGE_ORDER:
                cc_j = _TBL_PLANE[m][0]
                for si, sub in enumerate(STAGE_SUBS[m]):
                    nq, dq, de, s0, placement = sub
                    fd = nq * 2 * J
                    T = tupool.tile([P, fd], F16, tag="t")
                    U = tupool.tile([P, fd], F16, tag="u")
                    grid = [[dq * J, nq], [de * J, 2], [1, J]]
                    tu_out = [[2 * J, nq], [J, 2], [1, J]]
                    # T = X[grid] * c2
                    nc.vector.tensor_mul(
                        _ap(T[:], 0, tu_out),
                        _ap(X[:], s0 * J, grid),
                        _ap(TBL[:], cc_j * J, [[0, nq], [0, 2], [1, J]]))
                    # U = X[partner] * (tau-signed s2)
                    nc.vector.tensor_mul(
                        _ap(U[:], 0, tu_out),
                        _ap(X[:], (s0 + de) * J,
                            [[dq * J, nq], [-de * J, 2], [1, J]]),
                        ss_ap(TBL, m, sub))
                    # X[grid] = T + U ; last stage: split by e-halves so the
                    # first output DMA overlaps the second add
                    if m == STAGE_ORDER[-1]:
                        half = [[dq * J, nq], [1, J]]
                        tu_half = [[2 * J, nq], [1, J]]
                        nc.vector.tensor_add(
                            _ap(X[:], s0 * J, half),
                            _ap(T[:], 0, tu_half), _ap(U[:], 0, tu_half))
                        nc.sync.dma_start(out_d[:, 0:4 * J], X[:, 0:4 * J])
                        nc.vector.tensor_add(
                            _ap(X[:], (s0 + de) * J, half),
                            _ap(T[:], J, tu_half), _ap(U[:], J, tu_half))
                        nc.sync.dma_start(out_d[:, 10 * J:12 * J],
                                          X[:, 10 * J:12 * J])
                        nc.sync.dma_start(out_d[:, 12 * J:14 * J],
                                          X[:, 12 * J:14 * J])
                    else:
                        nc.vector.tensor_add(
                            _ap(X[:], s0 * J, grid),
                            _ap(T[:], 0, tu_out),
                            _ap(U[:], 0, tu_out))
                if m == 5:
                    a, b = EARLY_OUT
                    nc.sync.dma_start(out_d[:, a * J:b * J],
                                      X[:, a * J:b * J])

    nc.compile()
    return nc


_PROGRAM_CACHE = {}


def _get_program():
    if "p" not in _PROGRAM_CACHE:
        _PROGRAM_CACHE["p"] = _build_program()
    return _PROGRAM_CACHE["p"]


def _build_in_maps(x, pos, coefs, theta0, cayley):
    """Host-side: slot-permuted fp16 x + per-core sign tables."""
    _verify_layout(cayley)
    # full-length cos/sin tables per plane: angle = theta0[p, i] * coef_i
    ang = theta0.astype(np.float64) * np.asarray(coefs, np.float64)[None, :]
    ctab = np.cos(ang).astype(np.float16)          # (MAX_LEN, 4)
    stab = np.sin(ang).astype(np.float16)
    plane_idx = {m: PLANE_BLADES.index(m) for m in STAGE_ORDER}

    pos_i = np.clip(pos, 0, MAX_LEN - 1).astype(np.int64)
    comp_order = [SLOT_TO_COMP[s] for s in range(NSLOT)]

    in_maps = []
    for g in range(NCORES):
        xr = np.ascontiguousarray(
            x[g * ROWS_PER_CORE:(g + 1) * ROWS_PER_CORE]
        ).reshape(P, J, MV)
        xs = xr.transpose(0, 2, 1)[:, comp_order, :]       # (P, 14, J)
        x16 = np.ascontiguousarray(xs).astype(np.float16).reshape(P, NSLOT * J)

        pg = pos_i[g * ROWS_PER_CORE:(g + 1) * ROWS_PER_CORE].reshape(P, J)
        tbl = np.empty((P, TBL_J, J), dtype=np.float16)
        for m in STAGE_ORDER:
            cc, ss, nrows = _TBL_PLANE[m]
            i = plane_idx[m]
            c2 = ctab[pg, i]                               # (P, J)
            s2 = stab[pg, i]
            tbl[:, cc, :] = c2
            if nrows == 2:
                tbl[:, ss, :] = s2
                tbl[:, ss + 1, :] = -s2
            else:                      # m5 pattern (+,-): rows s,-s,-s,s
                tbl[:, ss, :] = s2
                tbl[:, ss + 1, :] = -s2
                tbl[:, ss + 2, :] = -s2
                tbl[:, ss + 3, :] = s2
        in_maps.append({"x": x16, "tbl": tbl.reshape(P, TBL_J * J)})
    return in_maps


def kernel(x, pos, bx, by, bz, bw, theta, cayley, biv_mask, scalar_mask):
    x = np.asarray(x, dtype=np.float32)
    pos = np.asarray(pos)
    theta = np.asarray(theta, dtype=np.float32)
    cayley = np.asarray(cayley, dtype=np.float32)
    assert x.shape == (B, L, MV) and pos.shape == (B, L)

    coefs = [float(np.asarray(c, dtype=np.float32).reshape(MV)[b])
             for c, b in zip((bx, by, bz, bw), PLANE_BLADES)]
    theta0 = theta.reshape(MAX_LEN, 4)

    nc = _get_program()
    in_maps = _build_in_maps(x, pos, coefs, theta0, cayley)
    res = run_bass_kernel_spmd(nc, in_maps, core_ids=list(range(NCORES)))

    out = np.empty((B, L, MV), dtype=np.float32)
    comp_order = [SLOT_TO_COMP[s] for s in range(NSLOT)]
    for g in range(NCORES):
        r = res.results[g]["out"].reshape(P, NSLOT, J).astype(np.float32)
        og = np.empty((P, MV, J), dtype=np.float32)
        og[:, comp_order, :] = r
        xr = np.ascontiguousarray(
            x[g * ROWS_PER_CORE:(g + 1) * ROWS_PER_CORE]).reshape(P, J, MV)
        og[:, 0, :] = xr[:, :, 0]
        og[:, 15, :] = xr[:, :, 15]
        out[g * ROWS_PER_CORE:(g + 1) * ROWS_PER_CORE] = \
            og.transpose(0, 2, 1).reshape(ROWS_PER_CORE, L, MV)
    return out


# revision 14
# speedup vs baseline: 1.0443x; 1.0443x over previous
"""Trainium2 Bass kernel for CARE position encoding (rotor sandwich).

out = R x R~ factorizes into 4 sequential Givens stages (blades 6,9,5,3).
This implementation:
  - computes all cos/sin tables on the HOST (from pos/theta/coefs) and
    ships them as fp16 -- the device does zero transcendental work;
  - stores x per-core in a position-innermost "slot" layout
    X[partition, slot*J + j] (J=256 positions per partition, 14 slots;
    multivector components 0 and 15 are invariant and bypass the device);
  - each Givens stage is 3 (or 6) DVE tensor_tensor ops in fp16, whose
    access patterns have unit-stride 256-long innermost runs -> the DVE
    runs them in 2x_1P packed mode (verified on HW);
  - the slot permutation was chosen so every stage's pair structure is an
    affine "grid" slot(q,e) = s0 + dq*q + de*e expressible in <=3 free
    AP dims (planes 6,3 as one op-triple; planes 9,5 as two halves).

Sign conventions (tau = Cayley sign of the rotated pair) are baked into
per-sub sign tables SS[r*J + j], r = q + nq*e, so arbitrary per-pair
orientations are free.
"""
import numpy as np

import concourse.bass as bass
import concourse.tile as tile
from concourse import bacc, mybir
from concourse.bass_utils import run_bass_kernel_spmd

F16 = mybir.dt.float16
F32 = mybir.dt.float32

P = 128
NCORES = 8
B, L, MV = 16, 16384, 16
MAX_LEN = 16384
ROWS_PER_CORE = B // NCORES          # 2
N = ROWS_PER_CORE * L                # 32768 positions per core
J = N // P                           # 256 positions per partition
NSLOT = 14

PLANE_BLADES = (3, 5, 9, 6)          # reference order (stage order reversed)
STAGE_ORDER = (6, 9, 5, 3)           # innermost rotor applied first

# slot[comp] for comps 1..14 (0 and 15 bypass the device entirely)
SLOT = {1: 3, 2: 13, 3: 9, 4: 6, 5: 2, 6: 12, 7: 5, 8: 8,
        9: 1, 10: 11, 11: 7, 12: 4, 13: 0, 14: 10}
COMPS = [c for c in range(MV) if c not in (0, 15)]
SLOT_TO_COMP = {s: c for c, s in SLOT.items()}

# Per-stage sub-ops: (nq, dq, de, s0, placement) with
# slot(comp placement[q][e]) = s0 + dq*q + de*e ; validated vs Cayley below.
STAGE_SUBS = {
    6: [(4, -2, 7, 6, ((4, 2), (12, 10), (5, 3), (13, 11)))],
    9: [(2, 2, 5, 3, ((1, 8), (7, 14))),
        (2, -7, -2, 11, ((10, 3), (12, 5)))],
    5: [(2, 6, 3, 3, ((1, 4), (3, 6))),
        (2, 6, 3, 1, ((9, 12), (11, 14)))],
    3: [(4, -1, 10, 3, ((1, 2), (5, 6), (9, 10), (13, 14)))],
}

# Table layout (units of J elements per partition), one CC + one shared SS
# per PLANE.  SS rows: m6/m9/m3 uniform-tau -> 2 rows [+s2, -s2]; m5 mixed
# pattern (+,-) per half -> 4 rows [s2, -s2, -s2, s2] (r = 2q + e).
# Stage order: m6 (3J) | m9 (3J) | m5 (5J) | m3 (3J) = 14J total.
_TBL_PLANE = {6: (0, 1, 2), 9: (3, 4, 2), 5: (6, 7, 4), 3: (11, 12, 2)}
TBL_J = 14

# slots 4..9 are final after stage 5 (not touched by stage 3)
EARLY_OUT = (4, 10)                   # slot range [4, 10)
LATE_OUT = ((0, 4), (10, 14))


def _build_cayley(k=4):
    n = 1 << k
    C = np.zeros((n, n, n), dtype=np.float32)
    for a in range(n):
        for b in range(n):
            s, t = 0, a >> 1
            while t:
                s += bin(t & b).count("1")
                t >>= 1
            C[a, b, a ^ b] = -1.0 if (s & 1) else 1.0
    return C


def _verify_layout(cayley):
    """Check SLOT/STAGE_SUBS against the runtime Cayley tensor."""
    for m in STAGE_ORDER:
        rotated = set()
        for (nq, dq, de, s0, placement) in STAGE_SUBS[m]:
            for q, (a, b) in enumerate(placement):
                assert b == (a ^ m), (m, a, b)
                assert SLOT[a] == s0 + dq * q, (m, q, a)
                assert SLOT[b] == s0 + dq * q + de, (m, q, b)
                assert abs(cayley[a, m, b]) == 1.0
                rotated |= {a, b}
        expect = {c for c in range(MV) if bin(c & m).count("1") % 2 == 1}
        assert rotated == expect, (m, rotated, expect)


def _ap(base_ap, extra_off, dims):
    ap = [list(base_ap.ap[0])] + [list(d) for d in dims]
    return bass.AP(base_ap.tensor, base_ap.offset + extra_off, ap)


def _build_program():
    nc = bacc.Bacc("TRN2", target_bir_lowering=False, debug=False,
                   enable_asserts=False, num_devices=NCORES)
    x_d = nc.dram_tensor("x", [P, NSLOT * J], F16, kind="ExternalInput")
    t_d = nc.dram_tensor("tbl", [P, TBL_J * J], F16, kind="ExternalInput")
    out_d = nc.dram_tensor("out", [P, NSLOT * J], F16, kind="ExternalOutput")

    cayley = _build_cayley()

    def ss_ap(TBL, m, sub):
        nq, dq, de, s0, placement = sub
        ss_j = _TBL_PLANE[m][1]
        tau0 = [float(cayley[a, m, b]) for (a, b) in placement]
        if all(t == tau0[0] for t in tau0):
            t = tau0[0]
            off = ss_j * J + (0 if t > 0 else J)
            estep = J if t > 0 else -J
            return _ap(TBL[:], off, [[0, nq], [estep, 2], [1, J]])
        assert nq == 2 and tau0 == [1.0, -1.0], (m, tau0)
        return _ap(TBL[:], ss_j * J, [[2 * J, nq], [J, 2], [1, J]])

    with tile.TileContext(nc) as tc:
        with tc.tile_pool(name="data", bufs=1) as dpool, \
             tc.tile_pool(name="tu", bufs=1) as tupool:
            TBL = dpool.tile([P, TBL_J * J], F16)
            X = dpool.tile([P, NSLOT * J], F16)

            # spread input DMAs across idle engine queues so issues (and,
            # if the rings allow, transfers) run in parallel after the
            # framework preamble barrier
            nc.sync.dma_start(TBL[:, :3 * J], t_d[:, :3 * J])
            nc.sync.dma_start(X[:], x_d[:])
            nc.sync.dma_start(TBL[:, 3 * J:], t_d[:, 3 * J:])

            for m in STAGE_ORDER:
                cc_j = _TBL_PLANE[m][0]
                for si, sub in enumerate(STAGE_SUBS[m]):
                    nq, dq, de, s0, placement = sub
                    fd = nq * 2 * J
                    T = tupool.tile([P, fd], F16, tag="t")
                    U = tupool.tile([P, fd], F16, tag="u")
                    grid = [[dq * J, nq], [de * J, 2], [1, J]]
                    tu_out = [[2 * J, nq], [J, 2], [1, J]]
                    # T = X[grid] * c2
                    nc.vector.tensor_mul(
                        _ap(T[:], 0, tu_out),
                        _ap(X[:], s0 * J, grid),
                        _ap(TBL[:], cc_j * J, [[0, nq], [0, 2], [1, J]]))
                    # U = X[partner] * (tau-signed s2)
                    nc.vector.tensor_mul(
                        _ap(U[:], 0, tu_out),
                        _ap(X[:], (s0 + de) * J,
                            [[dq * J, nq], [-de * J, 2], [1, J]]),
                        ss_ap(TBL, m, sub))
                    # X[grid] = T + U ; last stage: split by e-halves so the
                    # first output DMA overlaps the second add
                    if m == STAGE_ORDER[-1]:
                        half = [[dq * J, nq], [1, J]]
                        tu_half = [[2 * J, nq], [1, J]]
                        nc.vector.tensor_add(
                            _ap(X[:], s0 * J, half),
                            _ap(T[:], 0, tu_half), _ap(U[:], 0, tu_half))
                        nc.sync.dma_start(out_d[:, 0:4 * J], X[:, 0:4 * J])
                        nc.vector.tensor_add(
                            _ap(X[:], (s0 + de) * J, half),
                            _ap(T[:], J, tu_half), _ap(U[:], J, tu_half))
                        nc.sync.dma_start(out_d[:, 10 * J:14 * J],
                                          X[:, 10 * J:14 * J])
                    else:
                        nc.vector.tensor_add(
                            _ap(X[:], s0 * J, grid),
                            _ap(T[:], 0, tu_out),
                            _ap(U[:], 0, tu_out))
                if m == 5:
                    a, b = EARLY_OUT
                    nc.sync.dma_start(out_d[:, a * J:b * J],
                                      X[:, a * J:b * J])

    nc.compile()
    return nc


_PROGRAM_CACHE = {}


def _get_program():
    if "p" not in _PROGRAM_CACHE:
        _PROGRAM_CACHE["p"] = _build_program()
    return _PROGRAM_CACHE["p"]


def _build_in_maps(x, pos, coefs, theta0, cayley):
    """Host-side: slot-permuted fp16 x + per-core sign tables."""
    _verify_layout(cayley)
    # full-length cos/sin tables per plane: angle = theta0[p, i] * coef_i
    ang = theta0.astype(np.float64) * np.asarray(coefs, np.float64)[None, :]
    ctab = np.cos(ang).astype(np.float16)          # (MAX_LEN, 4)
    stab = np.sin(ang).astype(np.float16)
    plane_idx = {m: PLANE_BLADES.index(m) for m in STAGE_ORDER}

    pos_i = np.clip(pos, 0, MAX_LEN - 1).astype(np.int64)
    comp_order = [SLOT_TO_COMP[s] for s in range(NSLOT)]

    in_maps = []
    for g in range(NCORES):
        xr = np.ascontiguousarray(
            x[g * ROWS_PER_CORE:(g + 1) * ROWS_PER_CORE]
        ).reshape(P, J, MV)
        xs = xr.transpose(0, 2, 1)[:, comp_order, :]       # (P, 14, J)
        x16 = np.ascontiguousarray(xs).astype(np.float16).reshape(P, NSLOT * J)

        pg = pos_i[g * ROWS_PER_CORE:(g + 1) * ROWS_PER_CORE].reshape(P, J)
        tbl = np.empty((P, TBL_J, J), dtype=np.float16)
        for m in STAGE_ORDER:
            cc, ss, nrows = _TBL_PLANE[m]
            i = plane_idx[m]
            c2 = ctab[pg, i]                               # (P, J)
            s2 = stab[pg, i]
            tbl[:, cc, :] = c2
            if nrows == 2:
                tbl[:, ss, :] = s2
                tbl[:, ss + 1, :] = -s2
            else:                      # m5 pattern (+,-): rows s,-s,-s,s
                tbl[:, ss, :] = s2
                tbl[:, ss + 1, :] = -s2
                tbl[:, ss + 2, :] = -s2
                tbl[:, ss + 3, :] = s2
        in_maps.append({"x": x16, "tbl": tbl.reshape(P, TBL_J * J)})
    return in_maps


def kernel(x, pos, bx, by, bz, bw, theta, cayley, biv_mask, scalar_mask):
    x = np.asarray(x, dtype=np.float32)
    pos = np.asarray(pos)
    theta = np.asarray(theta, dtype=np.float32)
    cayley = np.asarray(cayley, dtype=np.float32)
    assert x.shape == (B, L, MV) and pos.shape == (B, L)

    coefs = [float(np.asarray(c, dtype=np.float32).reshape(MV)[b])
             for c, b in zip((bx, by, bz, bw), PLANE_BLADES)]
    theta0 = theta.reshape(MAX_LEN, 4)

    nc = _get_program()
    in_maps = _build_in_maps(x, pos, coefs, theta0, cayley)
    res = run_bass_kernel_spmd(nc, in_maps, core_ids=list(range(NCORES)))

    out = np.empty((B, L, MV), dtype=np.float32)
    comp_order = [SLOT_TO_COMP[s] for s in range(NSLOT)]
    for g in range(NCORES):
        r = res.results[g]["out"].reshape(P, NSLOT, J).astype(np.float32)
        og = np.empty((P, MV, J), dtype=np.float32)
        og[:, comp_order, :] = r
        xr = np.ascontiguousarray(
            x[g * ROWS_PER_CORE:(g + 1) * ROWS_PER_CORE]).reshape(P, J, MV)
        og[:, 0, :] = xr[:, :, 0]
        og[:, 15, :] = xr[:, :, 15]
        out[g * ROWS_PER_CORE:(g + 1) * ROWS_PER_CORE] = \
            og.transpose(0, 2, 1).reshape(ROWS_PER_CORE, L, MV)
    return out
